# revision 2
# baseline (speedup 1.0000x reference)
"""GQA kernel for Trainium2, 8 NeuronCores (DP over batch x TP over heads).

Problem (hardcoded): B=4, S=1024, EMBED=2048, HEADS=32, GROUPS=8,
GROUP_HEADS=4, HEAD_DIM=64.

Sharding: core c handles batch b = c//2 and TP half m = c%2
(16 heads = 4 groups per core). All tensors are pre-transposed on the
host so the device only ever sees contract-dim-major operands:

  qT/kT/vT  [2048, 1024]   (embed-major tokens for one batch)
  wqT       [2048, 1024]   (Wq rows m*1024:(m+1)*1024, transposed, pre-scaled 1/8)
  wkT/wvT   [2048, 256]    (Wk/Wv rows m*256:(m+1)*256, transposed)
  wfcT      [1024, 2048]   (Wfc columns m*1024:(m+1)*1024, transposed)
  y         [1024, 2048]   partial output; host computes y[2b] + y[2b+1] + bfc.

Device pipeline per core (all matmuls fp32r):
  1. khT [256,1024] and vh [1024,256] projections; khT duplicated into
     per-group [128,1024] tiles (both 64-partition halves hold the same
     group) so score matmuls can run at either array quadrant; vh gets
     a ones column (AV matmul then emits softmax denominators for free).
  2. qhT [1024,1024] projection (head-dim-major).
  3. Per head: scores = khT_g.T @ qhT_h -> exp (ACT, no max subtraction:
     |score| <= ~6 by construction) -> AV accumulation (vh as stationary)
     -> normalize by denominator broadcast (DMA partition-replicate +
     DVE reciprocal/mul) into OT [1024,1024].
  4. y = OT.T @ wfcT accumulated over 8 i-chunks.
"""

import numpy as np

import concourse.bass as bass
import concourse.tile as tile
from concourse import bacc, mybir
from concourse.bass_utils import run_bass_kernel_spmd

F32 = mybir.dt.float32
F32R = mybir.dt.float32r
AF = mybir.ActivationFunctionType

B, S, E = 4, 1024, 2048
HEADS_L = 16          # heads per core
GROUPS_L = 4          # groups per core
D = 64                # head dim
P = 128
NE = E // P           # 16 e-chunks
NT = S // P           # 8 token chunks
HO = HEADS_L * D      # 1024 local head-dims
GO = GROUPS_L * D     # 256 local group-dims

_CACHE = {}


def _build():
    nc = bacc.Bacc("TRN2")
    qT = nc.declare_dram_parameter("qT", [E, S], F32R, isOutput=False)
    kT = nc.declare_dram_parameter("kT", [E, S], F32R, isOutput=False)
    vT = nc.declare_dram_parameter("vT", [E, S], F32R, isOutput=False)
    wqT = nc.declare_dram_parameter("wqT", [E, HO], F32R, isOutput=False)
    wkT = nc.declare_dram_parameter("wkT", [E, GO], F32R, isOutput=False)
    wvT = nc.declare_dram_parameter("wvT", [E, GO], F32R, isOutput=False)
    wfcT = nc.declare_dram_parameter("wfcT", [HO, E], F32R, isOutput=False)
    y = nc.declare_dram_parameter("y", [S, E], F32, isOutput=True)
    dbg = {}
    if _CACHE.get("debug"):
        for nm, shp in [("dqh", [P, S]), ("dkh", [P, S]), ("dvh", [P, GROUPS_L * (D + 1)]),
                        ("dexp", [P, S]), ("drecip", [P, S]), ("dot", [P, S])]:
            dbg[nm] = nc.declare_dram_parameter(nm, shp, F32, isOutput=True)

    with tile.TileContext(nc) as tc:
        _body(nc, tc, qT, kT, vT, wqT, wkT, wvT, wfcT, y, dbg)
    nc.finalize()
    return nc


def _body(nc, tc, qT, kT, vT, wqT, wkT, wvT, wfcT, y, dbg=None):
    dbg = dbg or {}
    from contextlib import ExitStack
    with ExitStack() as ctx:
        # persistent pools (whole kernel lifetime)
        p_kh = ctx.enter_context(tc.tile_pool(name="kh", bufs=GROUPS_L))
        p_vh = ctx.enter_context(tc.tile_pool(name="vh", bufs=NT))
        p_qh = ctx.enter_context(tc.tile_pool(name="qh", bufs=NT))
        p_ot = ctx.enter_context(tc.tile_pool(name="ot", bufs=NT))
        p_wfc = ctx.enter_context(tc.tile_pool(name="wfc", bufs=8))
        p_y = ctx.enter_context(tc.tile_pool(name="y", bufs=2))
        ps = ctx.enter_context(tc.tile_pool(name="ps", bufs=8, space="PSUM"))

        kh_dup = [p_kh.tile([P, S], F32R, tag="kh", name=f"khdup_{g}") for g in range(GROUPS_L)]
        vh_aug = [p_vh.tile([P, GROUPS_L, D + 1], F32R, tag="vh", name=f"vhaug_{t}")
                  for t in range(NT)]
        qh_t = [p_qh.tile([P, S], F32R, tag="qh", name=f"qh_{t}") for t in range(NT)]
        ot_t = [p_ot.tile([P, S], F32R, tag="ot", name=f"ot_{t}") for t in range(NT)]

        with tc.tile_pool(name="wk", bufs=NE) as p_wk, \
             tc.tile_pool(name="wv", bufs=NE) as p_wv, \
             tc.tile_pool(name="kv", bufs=4) as p_kv, \
             tc.tile_pool(name="wq", bufs=4) as p_wq:
            # ---- stage 1: K/V projections ---------------------------
            wk_t = []
            wv_t = []
            kh_ps = [[ps.tile([P, 512], F32, tag="ps", name=f"khps_{a}_{b}") for b in range(2)]
                     for a in range(2)]
            for e in range(NE):
                wkt = p_wk.tile([P, GO], F32R, tag="wk")
                nc.sync.dma_start(out=wkt, in_=wkT[e * P:(e + 1) * P, :])
                wk_t.append(wkt)
                kte = p_kv.tile([P, S], F32R, tag="kv", name=f"kte_{e}")
                nc.sync.dma_start(out=kte, in_=kT[e * P:(e + 1) * P, :])
                wvt = p_wv.tile([P, GO], F32R, tag="wv")
                nc.sync.dma_start(out=wvt, in_=wvT[e * P:(e + 1) * P, :])
                wv_t.append(wvt)
                for o2 in range(2):
                    for t2 in range(2):
                        nc.tensor.matmul(
                            kh_ps[o2][t2][:, :],
                            wk_t[e][:, o2 * P:(o2 + 1) * P],
                            kte[:, t2 * 512:(t2 + 1) * 512],
                            start=(e == 0), stop=(e == NE - 1),
                        )
            for o2 in range(2):
                for t2 in range(2):
                    sl = slice(t2 * 512, (t2 + 1) * 512)
                    nc.vector.tensor_copy(kh_dup[2 * o2][0:D, sl],
                                          kh_ps[o2][t2][0:D, :])
                    nc.vector.tensor_copy(kh_dup[2 * o2 + 1][D:P, sl],
                                          kh_ps[o2][t2][D:P, :])
            for g in range(GROUPS_L):
                if g % 2 == 0:
                    nc.gpsimd.dma_start(out=kh_dup[g][D:P, :], in_=kh_dup[g][0:D, :])
                else:
                    nc.gpsimd.dma_start(out=kh_dup[g][0:D, :], in_=kh_dup[g][D:P, :])

            vh_ps = [ps.tile([P, GO], F32, tag="ps", name=f"vhps_{t}") for t in range(NT)]
            for e in range(NE):
                vte = p_kv.tile([P, S], F32R, tag="kv", name=f"vte_{e}")
                nc.sync.dma_start(out=vte, in_=vT[e * P:(e + 1) * P, :])
                for t in range(NT):
                    nc.tensor.matmul(
                        vh_ps[t][:, :],
                        vte[:, t * P:(t + 1) * P],
                        wv_t[e][:, :],
                        start=(e == 0), stop=(e == NE - 1),
                    )
            for t in range(NT):
                for g in range(GROUPS_L):
                    nc.vector.tensor_copy(vh_aug[t][:, g, 0:D],
                                          vh_ps[t][:, g * D:(g + 1) * D])
                ones = nc.const_aps.tensor(1.0, (P, 1), F32)
                for g in range(GROUPS_L):
                    nc.vector.tensor_copy(vh_aug[t][:, g, D:D + 1], ones)

            # ---- stage 2: Q projection ------------------------------
            for rnd in range(2):
                wq_r = []
                for e in range(NE):
                    wqe = p_wq.tile([P, 512], F32R, tag="wq", name=f"wq_{rnd}_{e}")
                    nc.sync.dma_start(
                        out=wqe,
                        in_=wqT[e * P:(e + 1) * P, rnd * 512:(rnd + 1) * 512])
                    wq_r.append(wqe)
                qps = [[ps.tile([P, 512], F32, tag="ps", name=f"qps_{a}_{b}") for b in range(2)]
                       for a in range(4)]
                for e in range(NE):
                    qte = p_kv.tile([P, S], F32R, tag="kv", name=f"qte_{rnd}_{e}")
                    nc.sync.dma_start(out=qte, in_=qT[e * P:(e + 1) * P, :])
                    for o in range(4):
                        for t2 in range(2):
                            nc.tensor.matmul(
                                qps[o][t2][:, :],
                                wq_r[e][:, o * P:(o + 1) * P],
                                qte[:, t2 * 512:(t2 + 1) * 512],
                                start=(e == 0), stop=(e == NE - 1),
                            )
                for o in range(4):
                    for t2 in range(2):
                        nc.scalar.activation(
                            qh_t[rnd * 4 + o][:, t2 * 512:(t2 + 1) * 512],
                            qps[o][t2][:, :], AF.Copy)

        if dbg:
            nc.sync.dma_start(out=dbg["dqh"][:, :], in_=qh_t[0][:, :].bitcast(F32))
            nc.sync.dma_start(out=dbg["dkh"][:, :], in_=kh_dup[0][:, :].bitcast(F32))
            nc.sync.dma_start(out=dbg["dvh"][:, :], in_=vh_aug[0].rearrange("p g d -> p (g d)").bitcast(F32))

        # ---- stage 3: attention per head ----------------------------
        with tc.tile_pool(name="exp", bufs=10) as p_exp, \
             tc.tile_pool(name="sm", bufs=3) as p_sm:
            for h in range(HEADS_L):
                g = h // 4
                qtile = qh_t[h // 2]
                qb = (h % 2) * D  # partition base inside qh tile

                exp_t = [p_exp.tile([P, S], F32R, tag="exp", name=f"exp_{h}_{kc}") for kc in range(NT)]
                for kc in range(NT):
                    for q2 in range(2):
                        sps = ps.tile([P, 512], F32, tag="ps", name=f"sps_{h}_{kc}_{q2}")
                        nc.tensor.matmul(
                            sps[:, :],
                            kh_dup[g][qb:qb + D, kc * P:(kc + 1) * P],
                            qtile[qb:qb + D, q2 * 512:(q2 + 1) * 512],
                            start=True, stop=True,
                        )
                        nc.scalar.activation(
                            exp_t[kc][:, q2 * 512:(q2 + 1) * 512], sps[:, :],
                            AF.Exp)

                den = p_sm.tile([P, S], F32, tag="den", name=f"den_{h}")
                av_ps = []
                for q2 in range(2):
                    ops = ps.tile([P, 512], F32, tag="ps", name=f"avps_{h}_{q2}")
                    for kc in range(NT):
                        nc.tensor.matmul(
                            ops[0:D + 1, :],
                            vh_aug[kc][:, g, :],
                            exp_t[kc][:, q2 * 512:(q2 + 1) * 512],
                            start=(kc == 0), stop=(kc == NT - 1),
                        )
                    nc.vector.tensor_copy(den[D:D + 1, q2 * 512:(q2 + 1) * 512],
                                          ops[D:D + 1, :])
                    av_ps.append(ops)
                recip = p_sm.tile([P, S], F32, tag="recip", name=f"recip_{h}")
                nc.gpsimd.dma_start(out=den[0:1, :], in_=den[D:D + 1, :])
                nc.gpsimd.partition_broadcast(recip[0:D, :], den[0:1, :])
                nc.vector.reciprocal(recip[0:D, :], recip[0:D, :])
                if dbg and h == 0:
                    nc.sync.dma_start(out=dbg["dexp"][:, :], in_=exp_t[0][:, :].bitcast(F32))
                    nc.sync.dma_start(out=dbg["drecip"][:, :], in_=recip[:, :])
                if h % 2 == 0:
                    for q2 in range(2):
                        sl = slice(q2 * 512, (q2 + 1) * 512)
                        nc.vector.tensor_mul(ot_t[h // 2][0:D, sl],
                                             av_ps[q2][0:D, :], recip[0:D, sl])
                else:
                    tmp = p_sm.tile([P, S], F32R, tag="tmp", name=f"tmp_{h}")
                    for q2 in range(2):
                        sl = slice(q2 * 512, (q2 + 1) * 512)
                        nc.vector.tensor_mul(tmp[0:D, sl],
                                             av_ps[q2][0:D, :], recip[0:D, sl])
                    nc.gpsimd.dma_start(out=ot_t[h // 2][D:P, :], in_=tmp[0:D, :])

            if dbg:
                nc.sync.dma_start(out=dbg["dot"][:, :], in_=ot_t[0][:, :].bitcast(F32))

        # ---- stage 4: output projection (four out-quarter rounds) ---
        for r in range(4):
            wfc_t = []
            for i in range(NT):
                wfct = p_wfc.tile([P, 512], F32R, tag="wfc", name=f"wfc_{r}_{i}")
                nc.sync.dma_start(
                    out=wfct,
                    in_=wfcT[i * P:(i + 1) * P, r * 512:(r + 1) * 512])
                wfc_t.append(wfct)
            for t in range(NT):
                y_sb = p_y.tile([P, 512], F32, tag="y", name=f"ysb_{r}_{t}")
                yps = ps.tile([P, 512], F32, tag="ps", name=f"yps_{r}_{t}")
                for i in range(NT):
                    nc.tensor.matmul(
                        yps[:, :],
                        ot_t[i][:, t * P:(t + 1) * P],
                        wfc_t[i][:, r * 0:512],
                        start=(i == 0), stop=(i == NT - 1),
                    )
                nc.scalar.activation(y_sb[:, :], yps[:, :], AF.Copy)
                nc.sync.dma_start(out=y[t * P:(t + 1) * P, r * 512:(r + 1) * 512],
                                  in_=y_sb)


def _get_nc():
    if "nc" not in _CACHE:
        _CACHE["nc"] = _build()
    return _CACHE["nc"]


def _make_in_maps(inputs):
    q = np.asarray(inputs["q"], np.float32)
    k = np.asarray(inputs["k"], np.float32)
    v = np.asarray(inputs["v"], np.float32)
    Wq = np.asarray(inputs["Wq"], np.float32)
    Wk = np.asarray(inputs["Wk"], np.float32)
    Wv = np.asarray(inputs["Wv"], np.float32)
    Wfc = np.asarray(inputs["Wfc"], np.float32)

    qTb = [np.ascontiguousarray(q[b].T) for b in range(B)]
    kTb = [np.ascontiguousarray(k[b].T) for b in range(B)]
    vTb = [np.ascontiguousarray(v[b].T) for b in range(B)]
    wqTm = [np.ascontiguousarray((Wq[m * HO:(m + 1) * HO, :] / 8.0).T)
            for m in range(2)]
    wkTm = [np.ascontiguousarray(Wk[m * GO:(m + 1) * GO, :].T) for m in range(2)]
    wvTm = [np.ascontiguousarray(Wv[m * GO:(m + 1) * GO, :].T) for m in range(2)]
    wfcTm = [np.ascontiguousarray(Wfc[:, m * HO:(m + 1) * HO].T)
             for m in range(2)]

    in_maps = []
    for c in range(8):
        b, m = c // 2, c % 2
        in_maps.append({
            "qT": qTb[b], "kT": kTb[b], "vT": vTb[b],
            "wqT": wqTm[m], "wkT": wkTm[m], "wvT": wvTm[m],
            "wfcT": wfcTm[m],
        })
    return in_maps


def kernel(q, k, v, Wq, Wk, Wv, Wfc, bfc):
    bfc = np.asarray(bfc, np.float32)
    nc = _get_nc()
    in_maps = _make_in_maps({"q": q, "k": k, "v": v, "Wq": Wq, "Wk": Wk,
                             "Wv": Wv, "Wfc": Wfc})
    res = run_bass_kernel_spmd(nc, in_maps, list(range(8)))
    out = np.empty((B, S, E), np.float32)
    for b in range(B):
        out[b] = res.results[2 * b]["y"] + res.results[2 * b + 1]["y"] + bfc
    return out



# revision 6
# speedup vs baseline: 1.3264x; 1.3264x over previous
"""GQA kernel for Trainium2, 8 NeuronCores (DP over batch x TP over heads).

Problem (hardcoded): B=4, S=1024, EMBED=2048, HEADS=32, GROUPS=8,
GROUP_HEADS=4, HEAD_DIM=64.

Sharding: core c handles batch b = c//2 and TP half m = c%2
(16 heads = 4 groups per core). All device-side operands are bf16
(PE streams 1 row/cycle vs 2 for fp32r); PSUM accumulation is fp32.
Host pre-transposes so the device only sees contract-dim-major operands:

  qT/kT/vT  [2048, 1024] bf16  (embed-major tokens for one batch)
  wqT       [2048, 1024] bf16  (Wq rows m*1024:(m+1)*1024, T, pre-scaled 1/8)
  wkT/wvT   [2048, 256]  bf16  (Wk/Wv rows m*256:(m+1)*256, transposed)
  wfcT      [1024, 2048] bf16  (Wfc columns m*1024:(m+1)*1024, transposed)
  y         [1024, 2048] f32   partial; host computes y[2b] + y[2b+1] + bfc.

Device pipeline per core:
  P0  K and V projections interleaved per e-chunk. kh -> per-group
      [128,1024] bf16 tiles with the 64 group dims duplicated in both
      partition halves (so even/odd heads' score matmuls run at array
      row offsets 0/64 matching the qh pair layout). vh -> [128,4,66]
      bf16 per token chunk with ones columns at both ends ([1|vh|1]) so
      even heads use cols 1:66 ([vh|1], den lands at out partition 64)
      and odd heads cols 0:65 ([1|vh], den at out partition 0... even
      heads only; odd heads reuse [vh|1] at out base 0 then get shifted
      to partitions 64:128 after normalize).
  P1  Q projection, two 512-out-dim rounds (8 PSUM banks each).
  P2  Attention: score matmuls stream into rotating 3-bank PSUM group
      tiles ([128,1536]); one ACT Exp per group (amortizes the 352-cyc
      ACT instruction overhead) writes bf16 exp ring tiles; AV
      accumulates per head from ring slices with the vh ones column
      emitting softmax denominators for free; normalize = DVE
      reciprocal on the single den row + gpsimd partition broadcast +
      DVE multiply.
  P3  y = ot.T @ wfc accumulated over 8 i-chunks, 4 psum banks per
      token chunk, double buffered.
"""

import numpy as np
import ml_dtypes

import concourse.bass as bass
import concourse.tile as tile
from concourse import bacc, mybir
from concourse.bass_utils import run_bass_kernel_spmd

F32 = mybir.dt.float32
BF16 = mybir.dt.bfloat16
AF = mybir.ActivationFunctionType

B, S, E = 4, 1024, 2048
HEADS_L = 16          # heads per core
GROUPS_L = 4          # groups per core
D = 64                # head dim
P = 128
NE = E // P           # 16 e-chunks
NT = S // P           # 8 token chunks
HO = HEADS_L * D      # 1024 local head-dims
GO = GROUPS_L * D     # 256 local group-dims

NSLOT = HEADS_L * NT * 2      # 256 score slots of [128, 512]
GW = 3                        # slots per exp group (3 PSUM banks)
NEXP = 10                     # exp ring depth

_CACHE = {}


def _build():
    nc = bacc.Bacc("TRN2")
    qT = nc.declare_dram_parameter("qT", [E, S], BF16, isOutput=False)
    kT = nc.declare_dram_parameter("kT", [E, S], BF16, isOutput=False)
    vT = nc.declare_dram_parameter("vT", [E, S], BF16, isOutput=False)
    wqT = nc.declare_dram_parameter("wqT", [E, HO], BF16, isOutput=False)
    wkT = nc.declare_dram_parameter("wkT", [E, GO], BF16, isOutput=False)
    wvT = nc.declare_dram_parameter("wvT", [E, GO], BF16, isOutput=False)
    wfcT = nc.declare_dram_parameter("wfcT", [HO, E], BF16, isOutput=False)
    y = nc.declare_dram_parameter("y", [S, E], F32, isOutput=True)

    with tile.TileContext(nc) as tc:
        _body(nc, tc, qT, kT, vT, wqT, wkT, wvT, wfcT, y)
    nc.finalize()
    return nc


def _body(nc, tc, qT, kT, vT, wqT, wkT, wvT, wfcT, y):
    from contextlib import ExitStack
    with ExitStack() as ctx:
        # persistent SBUF pools
        p_kh = ctx.enter_context(tc.tile_pool(name="kh", bufs=GROUPS_L))
        p_vh = ctx.enter_context(tc.tile_pool(name="vh", bufs=NT))
        p_qh = ctx.enter_context(tc.tile_pool(name="qh", bufs=NT))
        p_ot = ctx.enter_context(tc.tile_pool(name="ot", bufs=NT))
        p_wfc = ctx.enter_context(tc.tile_pool(name="wfc", bufs=NT))
        p_exp = ctx.enter_context(tc.tile_pool(name="exp", bufs=NEXP))
        p_sm = ctx.enter_context(tc.tile_pool(name="sm", bufs=3))

        kh_dup = [p_kh.tile([P, S], BF16, tag="kh", name=f"khdup_{g}")
                  for g in range(GROUPS_L)]
        vh_aug = [p_vh.tile([P, GROUPS_L, D + 1], BF16, tag="vh",
                            name=f"vhaug_{t}") for t in range(NT)]
        qh_t = [p_qh.tile([P, S], BF16, tag="qh", name=f"qh_{t}")
                for t in range(NT)]
        ot_t = [p_ot.tile([P, S], BF16, tag="ot", name=f"ot_{t}")
                for t in range(NT)]
        ones = nc.const_aps.tensor(1.0, (P, 1), F32)
        warm = p_sm.tile([P, 8], F32, tag="warm", bufs=1, name="warm")
        nc.scalar.activation(warm[0:1, 0:1], ones[0:1, 0:1], AF.Exp)

        # ---- P0: K and V projections, interleaved per e-chunk --------
        with tc.tile_pool(name="wk", bufs=NE) as p_wk, \
             tc.tile_pool(name="wv", bufs=NE) as p_wv, \
             tc.tile_pool(name="kv", bufs=4) as p_kv, \
             tc.tile_pool(name="ps0", bufs=1, space="PSUM") as ps0:
            wk_t = []
            wv_t = []
            for e in range(NE):
                wkt = p_wk.tile([P, GO], BF16, tag="wk", name=f"wk_{e}")
                nc.sync.dma_start(out=wkt, in_=wkT[e * P:(e + 1) * P, :])
                wk_t.append(wkt)
                wvt = p_wv.tile([P, GO], BF16, tag="wv", name=f"wv_{e}")
                nc.sync.dma_start(out=wvt, in_=wvT[e * P:(e + 1) * P, :])
                wv_t.append(wvt)

            kh_ps = [[ps0.tile([P, 512], F32, tag="khps", bufs=4,
                               name=f"khps_{a}_{b}") for b in range(2)]
                     for a in range(2)]
            vh_ps = [ps0.tile([P, 512], F32, tag="vhps", bufs=4,
                              name=f"vhps_{tp}") for tp in range(4)]
            for e in range(NE):
                kte = p_kv.tile([P, S], BF16, tag="kv", name=f"kte_{e}")
                nc.sync.dma_start(out=kte, in_=kT[e * P:(e + 1) * P, :])
                vte = p_kv.tile([P, S], BF16, tag="kv", name=f"vte_{e}")
                nc.sync.dma_start(out=vte, in_=vT[e * P:(e + 1) * P, :])
                for o2 in range(2):
                    for t2 in range(2):
                        nc.tensor.matmul(
                            kh_ps[o2][t2][:, :],
                            wk_t[e][:, o2 * P:(o2 + 1) * P],
                            kte[:, t2 * 512:(t2 + 1) * 512],
                            start=(e == 0), stop=(e == NE - 1),
                        )
                for t in range(NT):
                    nc.tensor.matmul(
                        vh_ps[t // 2][:, (t % 2) * GO:(t % 2 + 1) * GO],
                        vte[:, t * P:(t + 1) * P],
                        wv_t[e][:, :],
                        start=(e == 0 and t % 2 == 0),
                        stop=(e == NE - 1 and t % 2 == 1),
                        skip_group_check=True,
                    )
            # kh -> bf16 per-group tiles (dup both halves)
            for o2 in range(2):
                for t2 in range(2):
                    sl = slice(t2 * 512, (t2 + 1) * 512)
                    nc.vector.tensor_copy(kh_dup[2 * o2][0:D, sl],
                                          kh_ps[o2][t2][0:D, :])
                    nc.vector.tensor_copy(kh_dup[2 * o2 + 1][D:P, sl],
                                          kh_ps[o2][t2][D:P, :])
            for g in range(GROUPS_L):
                if g % 2 == 0:
                    nc.gpsimd.dma_start(out=kh_dup[g][D:P, :],
                                        in_=kh_dup[g][0:D, :])
                else:
                    nc.gpsimd.dma_start(out=kh_dup[g][0:D, :],
                                        in_=kh_dup[g][D:P, :])
            # vh -> bf16 [1 | vh | 1] per (t, g)
            for t in range(NT):
                src = vh_ps[t // 2][:, (t % 2) * GO:(t % 2 + 1) * GO]
                for g in range(GROUPS_L):
                    nc.vector.tensor_copy(vh_aug[t][:, g, 0:D],
                                          src[:, g * D:(g + 1) * D])
                    nc.vector.tensor_copy(vh_aug[t][:, g, D:D + 1], ones)

        # ---- P1: Q projection (two 512-out-dim rounds) ---------------
        with tc.tile_pool(name="wq", bufs=NE) as p_wq, \
             tc.tile_pool(name="qte", bufs=NE) as p_qte, \
             tc.tile_pool(name="ps1", bufs=1, space="PSUM") as ps1:
            qte_t = []
            for e in range(NE):
                qte = p_qte.tile([P, S], BF16, tag="qte", name=f"qte_{e}")
                nc.sync.dma_start(out=qte, in_=qT[e * P:(e + 1) * P, :])
                qte_t.append(qte)
            for rnd in range(2):
                wq_t = []
                for e in range(NE):
                    wqe = p_wq.tile([P, 512], BF16, tag="wq",
                                    name=f"wq_{rnd}_{e}")
                    nc.sync.dma_start(
                        out=wqe,
                        in_=wqT[e * P:(e + 1) * P, rnd * 512:(rnd + 1) * 512])
                    wq_t.append(wqe)
                qps = [ps1.tile([P, 512], F32, tag="qps", bufs=8,
                                name=f"qps_{rnd}_{i}") for i in range(8)]
                for e in range(NE):
                    for o in range(4):
                        st = wq_t[e][:, o * P:(o + 1) * P]
                        for q2 in range(2):
                            nc.tensor.matmul(
                                qps[o * 2 + q2][:, :],
                                st,
                                qte_t[e][:, q2 * 512:(q2 + 1) * 512],
                                start=(e == 0), stop=(e == NE - 1),
                            )
                for o in range(4):
                    for q2 in range(2):
                        nc.vector.tensor_copy(
                            qh_t[rnd * 4 + o][:, q2 * 512:(q2 + 1) * 512],
                            qps[o * 2 + q2][:, :])

        # prefetch wfc during attention
        wfc_t = []
        for i in range(NT):
            wfct = p_wfc.tile([P, E], BF16, tag="wfc", name=f"wfc_{i}")
            nc.sync.dma_start(out=wfct, in_=wfcT[i * P:(i + 1) * P, :])
            wfc_t.append(wfct)

        # ---- P2: attention ------------------------------------------
        with tc.tile_pool(name="ps2", bufs=1, space="PSUM") as ps2:
            exp_tiles = {}
            sc_tile = None

            def emit_av(h):
                g = h // 4
                even = (h % 2 == 0)
                avs = []
                for q2 in range(2):
                    av = ps2.tile([P, 512], F32, tag="av", bufs=2,
                                  name=f"av_{h}_{q2}")
                    for kc in range(NT):
                        s2 = 16 * h + kc * 2 + q2
                        g2, off2 = s2 // GW, s2 % GW
                        et = exp_tiles[g2]
                        st = vh_aug[kc][:, g, 0:D + 1]
                        nc.tensor.matmul(
                            av[0:D + 1, :], st,
                            et[:, off2 * 512:(off2 + 1) * 512],
                            start=(kc == 0), stop=(kc == NT - 1),
                        )
                    avs.append(av)
                # normalize: recip on den row, broadcast, multiply
                rr = p_sm.tile([P, S], F32, tag="rr", bufs=2,
                               name=f"rr_{h}")
                for q2 in range(2):
                    nc.vector.tensor_copy(rr[D:D + 1, q2 * 512:(q2 + 1) * 512],
                                          avs[q2][D:D + 1, :])
                nc.gpsimd.dma_start(out=rr[0:1, :], in_=rr[D:D + 1, :])
                rd = p_sm.tile([P, S], F32, tag="rd", bufs=2,
                               name=f"rd_{h}")
                nc.vector.reciprocal_approx_fast(out=rd[0:1, :],
                                                 in_=rr[0:1, :])
                bc = p_sm.tile([P, S], F32, tag="bc", bufs=2,
                               name=f"bc_{h}")
                nc.gpsimd.partition_broadcast(bc[0:D, :], rd[0:1, :])
                if even:
                    for q2 in range(2):
                        sl = slice(q2 * 512, (q2 + 1) * 512)
                        nc.vector.tensor_mul(ot_t[h // 2][0:D, sl],
                                             avs[q2][0:D, :], bc[0:D, sl])
                else:
                    tmp = p_sm.tile([P, S], BF16, tag="tmp", bufs=2,
                                    name=f"tmp_{h}")
                    for q2 in range(2):
                        sl = slice(q2 * 512, (q2 + 1) * 512)
                        nc.vector.tensor_mul(tmp[0:D, sl],
                                             avs[q2][0:D, :], bc[0:D, sl])
                    nc.gpsimd.dma_start(out=ot_t[h // 2][D:P, :],
                                        in_=tmp[0:D, :])

            for s in range(NSLOT):
                h, kc, q2 = s // 16, (s % 16) // 2, s % 2
                g = h // 4
                qb = (h % 2) * D
                qtile = qh_t[h // 2]
                if s % GW == 0:
                    sc_tile = ps2.tile([P, GW * 512], F32, tag="sc", bufs=2,
                                       name=f"sc_{s // GW}")
                off = s % GW
                nc.tensor.matmul(
                    sc_tile[:, off * 512:(off + 1) * 512],
                    kh_dup[g][qb:qb + D, kc * P:(kc + 1) * P],
                    qtile[qb:qb + D, q2 * 512:(q2 + 1) * 512],
                    start=True, stop=True,
                )
                if off == GW - 1 or s == NSLOT - 1:
                    gi = s // GW
                    w = (off + 1) * 512
                    et = p_exp.tile([P, GW * 512], BF16, tag="exp",
                                    name=f"exp_{gi}")
                    nc.scalar.activation(et[:, 0:w], sc_tile[:, 0:w], AF.Exp)
                    exp_tiles[gi] = et
                    for h2 in range(HEADS_L):
                        if (16 * h2 + 15) // GW == gi:
                            emit_av(h2)

        # ---- P3: output projection ----------------------------------
        with tc.tile_pool(name="ysb", bufs=4) as p_ysb, \
             tc.tile_pool(name="ps3", bufs=1, space="PSUM") as ps3:
            for t in range(NT):
                yt = [ps3.tile([P, 512], F32, tag="yps", bufs=8,
                               name=f"yps_{t}_{r}") for r in range(4)]
                for i in range(NT):
                    st = ot_t[i][:, t * P:(t + 1) * P]
                    for r in range(4):
                        nc.tensor.matmul(
                            yt[r][:, :], st,
                            wfc_t[i][:, r * 512:(r + 1) * 512],
                            start=(i == 0), stop=(i == NT - 1),
                        )
                for r in range(4):
                    ys = p_ysb.tile([P, 512], F32, tag="ysb",
                                    name=f"ysb_{t}_{r}")
                    nc.scalar.activation(ys[:, :], yt[r][:, :], AF.Copy)
                    nc.sync.dma_start(
                        out=y[t * P:(t + 1) * P, r * 512:(r + 1) * 512],
                        in_=ys)


def _get_nc():
    if "nc" not in _CACHE:
        _CACHE["nc"] = _build()
    return _CACHE["nc"]


def _bf16(x):
    return np.ascontiguousarray(x.astype(ml_dtypes.bfloat16))


def _make_in_maps(inputs):
    q = np.asarray(inputs["q"], np.float32)
    k = np.asarray(inputs["k"], np.float32)
    v = np.asarray(inputs["v"], np.float32)
    Wq = np.asarray(inputs["Wq"], np.float32)
    Wk = np.asarray(inputs["Wk"], np.float32)
    Wv = np.asarray(inputs["Wv"], np.float32)
    Wfc = np.asarray(inputs["Wfc"], np.float32)

    qTb = [_bf16(q[b].T) for b in range(B)]
    kTb = [_bf16(k[b].T) for b in range(B)]
    vTb = [_bf16(v[b].T) for b in range(B)]
    wqTm = [_bf16((Wq[m * HO:(m + 1) * HO, :] / 8.0).T) for m in range(2)]
    wkTm = [_bf16(Wk[m * GO:(m + 1) * GO, :].T) for m in range(2)]
    wvTm = [_bf16(Wv[m * GO:(m + 1) * GO, :].T) for m in range(2)]
    wfcTm = [_bf16(Wfc[:, m * HO:(m + 1) * HO].T) for m in range(2)]

    in_maps = []
    for c in range(8):
        b, m = c // 2, c % 2
        in_maps.append({
            "qT": qTb[b], "kT": kTb[b], "vT": vTb[b],
            "wqT": wqTm[m], "wkT": wkTm[m], "wvT": wvTm[m],
            "wfcT": wfcTm[m],
        })
    return in_maps


def kernel(q, k, v, Wq, Wk, Wv, Wfc, bfc):
    bfc = np.asarray(bfc, np.float32)
    nc = _get_nc()
    in_maps = _make_in_maps({"q": q, "k": k, "v": v, "Wq": Wq, "Wk": Wk,
                             "Wv": Wv, "Wfc": Wfc})
    res = run_bass_kernel_spmd(nc, in_maps, list(range(8)))
    out = np.empty((B, S, E), np.float32)
    for b in range(B):
        out[b] = res.results[2 * b]["y"] + res.results[2 * b + 1]["y"] + bfc
    return out


# revision 9
# speedup vs baseline: 1.3786x; 1.0394x over previous
"""GQA kernel for Trainium2, 8 NeuronCores (DP over batch x TP over heads).

Problem (hardcoded): B=4, S=1024, EMBED=2048, HEADS=32, GROUPS=8,
GROUP_HEADS=4, HEAD_DIM=64.

Sharding: core c handles batch b = c//2 and TP half m = c%2
(16 heads = 4 groups per core). All device-side operands are bf16
(PE streams 1 row/cycle vs 2 for fp32r); PSUM accumulation is fp32.
Host pre-transposes so the device only sees contract-dim-major operands:

  qT/kT/vT  [2048, 1024] bf16  (embed-major tokens for one batch)
  wqT       [2048, 1024] bf16  (Wq rows m*1024:(m+1)*1024, T, pre-scaled 1/8)
  wkT/wvT   [2048, 256]  bf16  (Wk/Wv rows m*256:(m+1)*256, transposed)
  wfcT      [1024, 2048] bf16  (Wfc columns m*1024:(m+1)*1024, transposed)
  y         [1024, 2048] f32   partial; host computes y[2b] + y[2b+1] + bfc.

Inputs stream in few, large, multi-chunk DMA descriptors spread across
engine queues (sync/vector/scalar/gpsimd) — single-queue descriptor
issue costs ~0.6us each and serializes.

Device pipeline per core:
  P0  K and V projections interleaved per e-chunk. kh -> per-group
      [128,1024] bf16 tiles with the 64 group dims duplicated in both
      partition halves. vh -> [128,4,65] bf16 ([vh|1]; the ones column
      makes the AV matmul emit softmax denominators at out partition 64
      for free). vh PSUM packs two token chunks per bank: start=True
      only on the first matmul touching a bank (start clears the whole
      bank), skip_group_check for the interleaved groups.
  P1  Q projection, two 512-out-dim rounds (8 PSUM banks each).
  P2  Attention: score matmuls stream into rotating 3-bank PSUM group
      tiles ([128,1536]); one ACT Exp per group (amortizes the ~352-cyc
      ACT instruction overhead) writes bf16 exp ring tiles; AV
      accumulates per head from ring slices (kc outer / q2 inner so the
      vh stationary is reused); normalize = fast-approx reciprocal on
      the single den row + gpsimd partition broadcast + DVE multiply;
      odd heads write via tmp + gpsimd DMA to partitions 64:128
      (matmul out base partition must be in {0,64}).
  P3  y = ot.T @ wfc accumulated over 8 i-chunks, 4 psum banks per
      token chunk, double buffered; y staged to [128,1024] tiles,
      two output DMAs per token chunk.
"""

import os

import numpy as np
import ml_dtypes

import concourse.bass as bass
import concourse.tile as tile
from concourse import bacc, mybir
from concourse.bass_utils import run_bass_kernel_spmd

if os.environ.get("BASS_LDW_OPT") == "1":
    import concourse.bass_utils as _bu

    if not getattr(_bu, "_ldw_opt_patched", False):
        _orig_run_command = _bu.run_command

        def _run_command_ldw(argv, **kwargs):
            argv = ["--enable-ldw-opt=true" if a == "--enable-ldw-opt=false"
                    else a for a in argv]
            return _orig_run_command(argv, **kwargs)

        _bu.run_command = _run_command_ldw
        _bu._ldw_opt_patched = True

F32 = mybir.dt.float32
BF16 = mybir.dt.bfloat16
AF = mybir.ActivationFunctionType

B, S, E = 4, 1024, 2048
HEADS_L = 16          # heads per core
GROUPS_L = 4          # groups per core
D = 64                # head dim
P = 128
NE = E // P           # 16 e-chunks
NT = S // P           # 8 token chunks
HO = HEADS_L * D      # 1024 local head-dims
GO = GROUPS_L * D     # 256 local group-dims

NSLOT = HEADS_L * NT * 2      # 256 score slots of [128, 512]
GW = 3                        # slots per exp group (3 PSUM banks)
NEXP = 10                     # exp ring depth

_CACHE = {}


def _build():
    nc = bacc.Bacc("TRN2")
    qT = nc.declare_dram_parameter("qT", [E, S], BF16, isOutput=False)
    kT = nc.declare_dram_parameter("kT", [E, S], BF16, isOutput=False)
    vT = nc.declare_dram_parameter("vT", [E, S], BF16, isOutput=False)
    wqT = nc.declare_dram_parameter("wqT", [E, HO], BF16, isOutput=False)
    wkT = nc.declare_dram_parameter("wkT", [E, GO], BF16, isOutput=False)
    wvT = nc.declare_dram_parameter("wvT", [E, GO], BF16, isOutput=False)
    wfcT = nc.declare_dram_parameter("wfcT", [HO, E], BF16, isOutput=False)
    y = nc.declare_dram_parameter("y", [S, E], F32, isOutput=True)

    with tile.TileContext(nc) as tc:
        _body(nc, tc, qT, kT, vT, wqT, wkT, wvT, wfcT, y)
    nc.finalize()
    return nc


def _body(nc, tc, qT, kT, vT, wqT, wkT, wvT, wfcT, y):
    from contextlib import ExitStack
    with ExitStack() as ctx:
        # persistent SBUF pools
        p_kh = ctx.enter_context(tc.tile_pool(name="kh", bufs=GROUPS_L))
        p_vh = ctx.enter_context(tc.tile_pool(name="vh", bufs=NT))
        p_qh = ctx.enter_context(tc.tile_pool(name="qh", bufs=NT))
        p_ot = ctx.enter_context(tc.tile_pool(name="ot", bufs=NT))
        p_wfc = ctx.enter_context(tc.tile_pool(name="wfc", bufs=1))
        p_exp = ctx.enter_context(tc.tile_pool(name="exp", bufs=NEXP))
        p_sm = ctx.enter_context(tc.tile_pool(name="sm", bufs=2))
        p_ysb = ctx.enter_context(tc.tile_pool(name="ysb", bufs=2))

        kh_dup = [p_kh.tile([P, S], BF16, tag="kh", name=f"khdup_{g}")
                  for g in range(GROUPS_L)]
        vh_aug = [p_vh.tile([P, GROUPS_L, D + 1], BF16, tag="vh",
                            name=f"vhaug_{t}") for t in range(NT)]
        qh_t = [p_qh.tile([P, S], BF16, tag="qh", name=f"qh_{t}")
                for t in range(NT)]
        ot_t = [p_ot.tile([P, S], BF16, tag="ot", name=f"ot_{t}")
                for t in range(NT)]
        ones = nc.const_aps.tensor(1.0, (P, 1), F32)
        warm = p_sm.tile([P, 8], F32, tag="warm", bufs=1, name="warm")
        nc.scalar.activation(warm[0:1, 0:1], ones[0:1, 0:1], AF.Exp)

        # ---- P0: K and V projections, interleaved per e-chunk --------
        with tc.tile_pool(name="wkv", bufs=1) as p_wkv, \
             tc.tile_pool(name="kv", bufs=2) as p_kv, \
             tc.tile_pool(name="ps0", bufs=1, space="PSUM") as ps0:
            wk_all = p_wkv.tile([P, NE, GO], BF16, tag="wk", name="wk_all")
            nc.gpsimd.dma_start(
                out=wk_all, in_=wkT.rearrange("(e p) g -> p e g", p=P))
            wv_all = p_wkv.tile([P, NE, GO], BF16, tag="wv", name="wv_all")
            nc.gpsimd.dma_start(
                out=wv_all, in_=wvT.rearrange("(e p) g -> p e g", p=P))

            kh_ps = [[ps0.tile([P, 512], F32, tag="khps", bufs=4,
                               name=f"khps_{a}_{b}") for b in range(2)]
                     for a in range(2)]
            vh_ps = [ps0.tile([P, 512], F32, tag="vhps", bufs=4,
                              name=f"vhps_{tp}") for tp in range(4)]
            for quarter in range(4):
                e0 = quarter * 4
                kq = p_kv.tile([P, 4, S], BF16, tag="kq", name=f"kq_{quarter}")
                nc.sync.dma_start(
                    out=kq,
                    in_=kT[e0 * P:(e0 + 4) * P, :].rearrange(
                        "(e p) s -> p e s", p=P))
                vq = p_kv.tile([P, 4, S], BF16, tag="vq", name=f"vq_{quarter}")
                nc.scalar.dma_start(
                    out=vq,
                    in_=vT[e0 * P:(e0 + 4) * P, :].rearrange(
                        "(e p) s -> p e s", p=P))
                for el in range(4):
                    e = e0 + el
                    for o2 in range(2):
                        for t2 in range(2):
                            nc.tensor.matmul(
                                kh_ps[o2][t2][:, :],
                                wk_all[:, e, o2 * P:(o2 + 1) * P],
                                kq[:, el, t2 * 512:(t2 + 1) * 512],
                                start=(e == 0), stop=(e == NE - 1),
                            )
                    for t in range(NT):
                        nc.tensor.matmul(
                            vh_ps[t // 2][:, (t % 2) * GO:(t % 2 + 1) * GO],
                            vq[:, el, t * P:(t + 1) * P],
                            wv_all[:, e, :],
                            start=(e == 0 and t % 2 == 0),
                            stop=(e == NE - 1 and t % 2 == 1),
                            skip_group_check=True,
                        )
            # kh -> bf16 per-group tiles (dup both halves)
            for o2 in range(2):
                for t2 in range(2):
                    sl = slice(t2 * 512, (t2 + 1) * 512)
                    nc.vector.tensor_copy(kh_dup[2 * o2][0:D, sl],
                                          kh_ps[o2][t2][0:D, :])
                    nc.vector.tensor_copy(kh_dup[2 * o2 + 1][D:P, sl],
                                          kh_ps[o2][t2][D:P, :])
            for g in range(GROUPS_L):
                if g % 2 == 0:
                    nc.gpsimd.dma_start(out=kh_dup[g][D:P, :],
                                        in_=kh_dup[g][0:D, :])
                else:
                    nc.gpsimd.dma_start(out=kh_dup[g][0:D, :],
                                        in_=kh_dup[g][D:P, :])
            # vh -> bf16 [vh | 1] per (t, g)
            for t in range(NT):
                src = vh_ps[t // 2][:, (t % 2) * GO:(t % 2 + 1) * GO]
                for g in range(GROUPS_L):
                    nc.vector.tensor_copy(vh_aug[t][:, g, 0:D],
                                          src[:, g * D:(g + 1) * D])
                    nc.vector.tensor_copy(vh_aug[t][:, g, D:D + 1], ones)

        # ---- P1: Q projection (two 512-out-dim rounds) ---------------
        with tc.tile_pool(name="wq", bufs=2) as p_wq, \
             tc.tile_pool(name="qte", bufs=4) as p_qte, \
             tc.tile_pool(name="ps1", bufs=1, space="PSUM") as ps1:
            qte_t = []
            for quarter in range(4):
                e0 = quarter * 4
                qq = p_qte.tile([P, 4, S], BF16, tag="qte",
                                name=f"qte_{quarter}")
                nc.sync.dma_start(
                    out=qq,
                    in_=qT[e0 * P:(e0 + 4) * P, :].rearrange(
                        "(e p) s -> p e s", p=P))
                qte_t.append(qq)
            for rnd in range(2):
                wq_t = []
                for half in range(2):
                    e0 = half * 8
                    wqe = p_wq.tile([P, 8, 512], BF16, tag="wq",
                                    name=f"wq_{rnd}_{half}")
                    nc.scalar.dma_start(
                        out=wqe,
                        in_=wqT[e0 * P:(e0 + 8) * P,
                                rnd * 512:(rnd + 1) * 512].rearrange(
                            "(e p) c -> p e c", p=P))
                    wq_t.append(wqe)
                qps = [ps1.tile([P, 512], F32, tag="qps", bufs=8,
                                name=f"qps_{rnd}_{i}") for i in range(8)]
                for e in range(NE):
                    for o in range(4):
                        st = wq_t[e // 8][:, e % 8, o * P:(o + 1) * P]
                        for q2 in range(2):
                            nc.tensor.matmul(
                                qps[o * 2 + q2][:, :],
                                st,
                                qte_t[e // 4][:, e % 4,
                                              q2 * 512:(q2 + 1) * 512],
                                start=(e == 0), stop=(e == NE - 1),
                            )
                for o in range(4):
                    for q2 in range(2):
                        nc.vector.tensor_copy(
                            qh_t[rnd * 4 + o][:, q2 * 512:(q2 + 1) * 512],
                            qps[o * 2 + q2][:, :])

        # prefetch wfc during attention (single descriptor, scalar queue)
        wfc_all = p_wfc.tile([P, NT, E], BF16, tag="wfc", name="wfc_all")
        nc.scalar.dma_start(
            out=wfc_all, in_=wfcT.rearrange("(i p) e -> p i e", p=P))

        # ---- P2: attention ------------------------------------------
        with tc.tile_pool(name="ps2", bufs=1, space="PSUM") as ps2:
            exp_tiles = {}
            sc_tile = None

            def emit_av(h):
                g = h // 4
                even = (h % 2 == 0)
                avs = [ps2.tile([P, 512], F32, tag="av", bufs=2,
                                name=f"av_{h}_{q2}") for q2 in range(2)]
                for kc in range(NT):
                    st = vh_aug[kc][:, g, 0:D + 1]
                    for q2 in range(2):
                        s2 = 16 * h + kc * 2 + q2
                        g2, off2 = s2 // GW, s2 % GW
                        et = exp_tiles[g2]
                        nc.tensor.matmul(
                            avs[q2][0:D + 1, :], st,
                            et[:, off2 * 512:(off2 + 1) * 512],
                            start=(kc == 0), stop=(kc == NT - 1),
                        )
                # normalize: fast recip on den row, broadcast, multiply
                rr = p_sm.tile([P, S], F32, tag="rr", name=f"rr_{h}")
                for q2 in range(2):
                    nc.vector.tensor_copy(rr[D:D + 1, q2 * 512:(q2 + 1) * 512],
                                          avs[q2][D:D + 1, :])
                nc.gpsimd.dma_start(out=rr[0:1, :], in_=rr[D:D + 1, :])
                rd = p_sm.tile([P, S], F32, tag="rd", name=f"rd_{h}")
                nc.vector.reciprocal_approx_fast(out=rd[0:1, :],
                                                 in_=rr[0:1, :])
                bc = p_sm.tile([P, S], F32, tag="bc", name=f"bc_{h}")
                nc.gpsimd.partition_broadcast(bc[0:D, :], rd[0:1, :])
                if even:
                    for q2 in range(2):
                        sl = slice(q2 * 512, (q2 + 1) * 512)
                        nc.vector.tensor_mul(ot_t[h // 2][0:D, sl],
                                             avs[q2][0:D, :], bc[0:D, sl])
                else:
                    tmp = p_sm.tile([P, S], BF16, tag="tmp", name=f"tmp_{h}")
                    for q2 in range(2):
                        sl = slice(q2 * 512, (q2 + 1) * 512)
                        nc.vector.tensor_mul(tmp[0:D, sl],
                                             avs[q2][0:D, :], bc[0:D, sl])
                    nc.gpsimd.dma_start(out=ot_t[h // 2][D:P, :],
                                        in_=tmp[0:D, :])

            for s in range(NSLOT):
                h, kc, q2 = s // 16, (s % 16) // 2, s % 2
                g = h // 4
                qb = (h % 2) * D
                qtile = qh_t[h // 2]
                if s % GW == 0:
                    sc_tile = ps2.tile([P, GW * 512], F32, tag="sc", bufs=2,
                                       name=f"sc_{s // GW}")
                off = s % GW
                nc.tensor.matmul(
                    sc_tile[:, off * 512:(off + 1) * 512],
                    kh_dup[g][qb:qb + D, kc * P:(kc + 1) * P],
                    qtile[qb:qb + D, q2 * 512:(q2 + 1) * 512],
                    start=True, stop=True,
                )
                if off == GW - 1 or s == NSLOT - 1:
                    gi = s // GW
                    w = (off + 1) * 512
                    et = p_exp.tile([P, GW * 512], BF16, tag="exp",
                                    name=f"exp_{gi}")
                    nc.scalar.activation(et[:, 0:w], sc_tile[:, 0:w], AF.Exp)
                    exp_tiles[gi] = et
                    for h2 in range(HEADS_L):
                        if (16 * h2 + 15) // GW == gi:
                            emit_av(h2)

        # ---- P3: output projection ----------------------------------
        with tc.tile_pool(name="ps3", bufs=1, space="PSUM") as ps3:
            for t in range(NT):
                yt = [ps3.tile([P, 512], F32, tag="yps", bufs=8,
                               name=f"yps_{t}_{r}") for r in range(4)]
                for i in range(NT):
                    st = ot_t[i][:, t * P:(t + 1) * P]
                    for r in range(4):
                        nc.tensor.matmul(
                            yt[r][:, :], st,
                            wfc_all[:, i, r * 512:(r + 1) * 512],
                            start=(i == 0), stop=(i == NT - 1),
                        )
                for r2 in range(2):
                    ys = p_ysb.tile([P, 1024], F32, tag="ysb",
                                    name=f"ysb_{t}_{r2}")
                    for r in range(2):
                        nc.scalar.activation(ys[:, r * 512:(r + 1) * 512],
                                             yt[r2 * 2 + r][:, :], AF.Copy)
                    nc.sync.dma_start(
                        out=y[t * P:(t + 1) * P,
                              r2 * 1024:(r2 + 1) * 1024],
                        in_=ys)


def _get_nc():
    if "nc" not in _CACHE:
        _CACHE["nc"] = _build()
    return _CACHE["nc"]


def _bf16(x):
    return np.ascontiguousarray(x.astype(ml_dtypes.bfloat16))


def _make_in_maps(inputs):
    q = np.asarray(inputs["q"], np.float32)
    k = np.asarray(inputs["k"], np.float32)
    v = np.asarray(inputs["v"], np.float32)
    Wq = np.asarray(inputs["Wq"], np.float32)
    Wk = np.asarray(inputs["Wk"], np.float32)
    Wv = np.asarray(inputs["Wv"], np.float32)
    Wfc = np.asarray(inputs["Wfc"], np.float32)

    qTb = [_bf16(q[b].T) for b in range(B)]
    kTb = [_bf16(k[b].T) for b in range(B)]
    vTb = [_bf16(v[b].T) for b in range(B)]
    wqTm = [_bf16((Wq[m * HO:(m + 1) * HO, :] / 8.0).T) for m in range(2)]
    wkTm = [_bf16(Wk[m * GO:(m + 1) * GO, :].T) for m in range(2)]
    wvTm = [_bf16(Wv[m * GO:(m + 1) * GO, :].T) for m in range(2)]
    wfcTm = [_bf16(Wfc[:, m * HO:(m + 1) * HO].T) for m in range(2)]

    in_maps = []
    for c in range(8):
        b, m = c // 2, c % 2
        in_maps.append({
            "qT": qTb[b], "kT": kTb[b], "vT": vTb[b],
            "wqT": wqTm[m], "wkT": wkTm[m], "wvT": wvTm[m],
            "wfcT": wfcTm[m],
        })
    return in_maps


def kernel(q, k, v, Wq, Wk, Wv, Wfc, bfc):
    bfc = np.asarray(bfc, np.float32)
    nc = _get_nc()
    in_maps = _make_in_maps({"q": q, "k": k, "v": v, "Wq": Wq, "Wk": Wk,
                             "Wv": Wv, "Wfc": Wfc})
    res = run_bass_kernel_spmd(nc, in_maps, list(range(8)))
    out = np.empty((B, S, E), np.float32)
    for b in range(B):
        out[b] = res.results[2 * b]["y"] + res.results[2 * b + 1]["y"] + bfc
    return out
